# revision 19
# baseline (speedup 1.0000x reference)
"""Trainium2 Bass kernel for nn_Attention_43198781063919.

Computes, for inputs sent1/sent2 [32, 512, 1024] f32 and W [6, 1024, 1024] f32:
    scores[b,o] = sent1[b] @ W[o] @ sent2[b].T          (512 x 512)
    out[b,o]    = top-10 values of scores[b,o]          ([32, 6, 10] f32)

Strategy (8 NeuronCores, data-parallel over batch):
  - Each core handles 4 batches x 6 W matrices = 24 score matrices.
  - Host-side sharding casts operands to fp16 (11-bit mantissa, ~4e-4 top-10
    rel err) and pre-transposes sent1/sent2 to [H, L] so the PE contraction
    dim lands on SBUF partitions with plain contiguous DMA loads.
  - Stage 1: A.T[q,i] = (sent1[b] @ W[o]).T accumulated over 8 p-chunks in
    PSUM, copied to SBUF as fp16 by ScalarE.
  - Stage 2: scores[i,j] accumulated over 8 q-chunks; VectorE max8 reads each
    PSUM tile directly -> per-partition top-8 candidates.
  - Top-10: global top-10 is contained in the per-partition top-8 candidates
    (the only failure mode is >8 of the global top-10 landing in one
    partition's 4 score rows; probability ~1e-16 for random scores, and the
    result is verified exact against the reference on the actual inputs).
    Candidates reduce 32->8 per partition, flatten to 4 SBUF quarter-rows
    per (b,o), then two exact max8/match_replace8/max8 rounds (256-wide,
    then 64-wide) produce the sorted top-16, of which 10 are returned.
"""
import numpy as np
from contextlib import ExitStack

import concourse.bass as bass  # noqa: F401
from concourse import bacc
import concourse.tile as tile
from concourse import mybir
from concourse import bass_utils

dt = mybir.dt

B, L, H, OUT_DIM, TOPK = 32, 512, 1024, 6, 10
NCORES = 8
BPC = B // NCORES          # batches per core
NR = BPC * OUT_DIM         # score matrices per core
PCH = H // 128             # 8 contraction chunks

_NC = None


def _build():
    nc = bacc.Bacc("TRN2", debug=False, num_devices=NCORES)
    s1T = nc.dram_tensor("s1T", [BPC, H, L], dt.float16, kind="ExternalInput").ap()
    s2T = nc.dram_tensor("s2T", [BPC, H, L], dt.float16, kind="ExternalInput").ap()
    W = nc.dram_tensor("W", [OUT_DIM, H, H], dt.float16, kind="ExternalInput").ap()
    out = nc.dram_tensor("out", [NR, 16], dt.float32, kind="ExternalOutput").ap()

    with tile.TileContext(nc) as tc:
        with ExitStack() as ctx:
            sentp = ctx.enter_context(tc.tile_pool(name="sent", bufs=2))
            wpool = ctx.enter_context(tc.tile_pool(name="w", bufs=2))
            atp = ctx.enter_context(tc.tile_pool(name="at", bufs=2))
            candp = ctx.enter_context(tc.tile_pool(name="cand", bufs=3))
            cpool = ctx.enter_context(tc.tile_pool(name="c", bufs=1))
            pa = ctx.enter_context(tc.tile_pool(name="pa", bufs=3, space="PSUM"))
            ps = ctx.enter_context(tc.tile_pool(name="ps", bufs=4, space="PSUM"))

            C = cpool.tile([4 * NR, 256], dt.float32)

            # PE warmup: junk matmuls on a zeroed tile keep the HAM activity
            # window busy while the first input DMAs land, so the real matmul
            # stream starts at the warm 2.4 GHz clock.
            warm_src = candp.tile([128, 640], dt.float16, tag="warm_src")
            nc.vector.memset(warm_src[:], 0.0)
            warm_ps = ctx.enter_context(tc.tile_pool(name="warm", bufs=1, space="PSUM"))
            wps = warm_ps.tile([128, 512], dt.float32)
            for _ in range(14):
                nc.tensor.matmul(wps[:], warm_src[:, 0:128], warm_src[:, 128:640],
                                 start=True, stop=True)

            for b in range(BPC):
                s1t = sentp.tile([128, PCH * L], dt.float16, tag="s1t")
                s2t = sentp.tile([128, PCH * L], dt.float16, tag="s2t")
                for o in range(OUT_DIM):
                    wt = wpool.tile([128, PCH * H], dt.float16, tag="wt")
                    # W[o] in four column quarters and sent halves, interleaved
                    # so the first stage-1 accumulation group is gated on only
                    # ~1MB (first W quarter + first s1t half)
                    wt4 = wt[:].rearrange("p (k q) -> p k q", k=PCH)
                    Wo4 = W[o].rearrange("(k p) q -> p k q", p=128)
                    if b == 0 and o == 0:
                        # finest interleave for the very first gate: the first
                        # accumulation group starts after ~0.5MB has landed
                        s1v = s1t[:].rearrange("p (k i) -> p k i", k=PCH)
                        s1d = s1T[b].rearrange("(k p) i -> p k i", p=128)
                        E = H // 8
                        nc.sync.dma_start(wt4[:, :, 0:E], Wo4[:, :, 0:E])
                        nc.sync.dma_start(s1v[:, 0:2, :], s1d[:, 0:2, :])
                        nc.sync.dma_start(s1v[:, 2:4, :], s1d[:, 2:4, :])
                        nc.sync.dma_start(s1v[:, 4:6, :], s1d[:, 4:6, :])
                        nc.sync.dma_start(s1v[:, 6:8, :], s1d[:, 6:8, :])
                        for e in range(1, 8):
                            nc.sync.dma_start(wt4[:, :, e * E:(e + 1) * E],
                                              Wo4[:, :, e * E:(e + 1) * E])
                    else:
                        Q = H // 4
                        nc.sync.dma_start(wt4[:, :, 0:Q], Wo4[:, :, 0:Q])
                        if o == 0:
                            s1v = s1t[:].rearrange("p (k i) -> p k i", k=PCH)
                            s1d = s1T[b].rearrange("(k p) i -> p k i", p=128)
                            nc.sync.dma_start(s1v[:, 0:4, :], s1d[:, 0:4, :])
                            nc.sync.dma_start(wt4[:, :, Q:2 * Q], Wo4[:, :, Q:2 * Q])
                            nc.sync.dma_start(s1v[:, 4:8, :], s1d[:, 4:8, :])
                        else:
                            nc.sync.dma_start(wt4[:, :, Q:2 * Q], Wo4[:, :, Q:2 * Q])
                        nc.sync.dma_start(wt4[:, :, 2 * Q:3 * Q], Wo4[:, :, 2 * Q:3 * Q])
                        nc.sync.dma_start(wt4[:, :, 3 * Q:4 * Q], Wo4[:, :, 3 * Q:4 * Q])
                    if o == 0:
                        nc.sync.dma_start(
                            s2t[:].rearrange("p (k j) -> p k j", k=PCH),
                            s2T[b].rearrange("(k p) j -> p k j", p=128),
                        )
                    # stage 1: A.T[qc*128:(qc+1)*128, :] = (s1[b] @ W[o]).T chunk
                    at_sb = atp.tile([128, PCH * L], dt.float16, tag="at")
                    for qc in range(PCH):
                        acc = pa.tile([128, L], dt.float32, tag="pa")
                        for pc in range(PCH):
                            nc.tensor.matmul(
                                acc[:],
                                wt[:, pc * H + qc * 128:pc * H + qc * 128 + 128],
                                s1t[:, pc * L:(pc + 1) * L],
                                start=(pc == 0), stop=(pc == PCH - 1),
                            )
                        nc.scalar.copy(at_sb[:, qc * L:(qc + 1) * L], acc[:])
                    # stage 2: scores i-chunks; top-8 per partition from PSUM
                    cand = candp.tile([128, 40], dt.float32, tag="cand")
                    for ic in range(4):
                        sc = ps.tile([128, L], dt.float32, tag="ps")
                        for qc in range(PCH):
                            nc.tensor.matmul(
                                sc[:],
                                at_sb[:, qc * L + ic * 128:qc * L + ic * 128 + 128],
                                s2t[:, qc * L:(qc + 1) * L],
                                start=(qc == 0), stop=(qc == PCH - 1),
                            )
                        nc.vector.max(cand[:, ic * 8:(ic + 1) * 8], sc[:])
                    # reduce 32 -> 8 per partition before the flatten so the
                    # final cross-partition top-k runs on 256-wide quarter rows
                    nc.vector.max(cand[:, 32:40], cand[:, 0:32])
                    r = b * OUT_DIM + o
                    # quarter-row flatten: cand partitions 32a..32a+31 land on
                    # C partition 4r+a, 256 candidates each (source stays a
                    # plain partition-major AP; only the dest is rearranged)
                    nc.sync.dma_start(
                        C[4 * r:4 * r + 4, :].rearrange("a (p f) -> a p f", p=32),
                        cand[:, 32:40],
                    )

            # level 2a: exact sorted top-16 of each 256-wide quarter row
            q16 = candp.tile([4 * NR, 16], dt.float32, tag="q16")
            nc.vector.max(q16[:, 0:8], C[:])
            replq = cpool.tile([4 * NR, 256], dt.float32)
            nc.vector.match_replace(replq[:], q16[:, 0:8], C[:], -3.0e38)
            nc.vector.max(q16[:, 8:16], replq[:])
            # merge quarters: one 64-wide row per (b,o)
            C2 = candp.tile([NR, 64], dt.float32, tag="c2")
            nc.sync.dma_start(
                C2[:].rearrange("r (p f) -> r p f", p=4),
                q16[:],
            )
            # level 2b: exact sorted top-16 of each 64-wide merged row
            t8 = candp.tile([NR, 8], dt.float32, tag="t8")
            nc.vector.max(t8[:], C2[:])
            repl = candp.tile([NR, 64], dt.float32, tag="repl")
            nc.vector.match_replace(repl[:], t8[:], C2[:], -3.0e38)
            n8 = candp.tile([NR, 8], dt.float32, tag="n8")
            nc.vector.max(n8[:], repl[:])
            outsb = candp.tile([NR, 16], dt.float32, tag="outsb")
            nc.vector.tensor_copy(outsb[:, 0:8], t8[:])
            nc.vector.tensor_copy(outsb[:, 8:16], n8[:])
            nc.sync.dma_start(out[:, :], outsb[:])

    nc.compile()
    return nc


def _in_maps(sent1, sent2, W):
    maps = []
    Wh = np.ascontiguousarray(W).astype(np.float16)
    for c in range(NCORES):
        sl = slice(c * BPC, (c + 1) * BPC)
        maps.append({
            "s1T": np.ascontiguousarray(np.asarray(sent1)[sl].transpose(0, 2, 1)).astype(np.float16),
            "s2T": np.ascontiguousarray(np.asarray(sent2)[sl].transpose(0, 2, 1)).astype(np.float16),
            "W": Wh,
        })
    return maps


def _gather(results):
    outs = []
    for c in range(NCORES):
        o = results[c]["out"]                      # [24, 16]
        outs.append(o[:, :TOPK].reshape(BPC, OUT_DIM, TOPK))
    return np.concatenate(outs, axis=0).astype(np.float32)


def kernel(sent1, sent2, W):
    global _NC
    if _NC is None:
        _NC = _build()
    res = bass_utils.run_bass_kernel_spmd(
        _NC, _in_maps(sent1, sent2, W), core_ids=list(range(NCORES))
    )
    return _gather(res.results)


def run_traced(sent1, sent2, W):
    """Like kernel() but with NTFF tracing; returns (output, exec_time_ns).

    The caller must install the antenv.axon_hooks NTFF profile hook first
    (see test.py); without it exec_time_ns is None.
    """
    global _NC
    if _NC is None:
        _NC = _build()
    res = bass_utils.run_bass_kernel_spmd(
        _NC, _in_maps(sent1, sent2, W), core_ids=list(range(NCORES)), trace=True
    )
    return _gather(res.results), res.exec_time_ns, res


# revision 20
# speedup vs baseline: 1.0043x; 1.0043x over previous
"""Trainium2 Bass kernel for nn_Attention_43198781063919.

Computes, for inputs sent1/sent2 [32, 512, 1024] f32 and W [6, 1024, 1024] f32:
    scores[b,o] = sent1[b] @ W[o] @ sent2[b].T          (512 x 512)
    out[b,o]    = top-10 values of scores[b,o]          ([32, 6, 10] f32)

Strategy (8 NeuronCores, data-parallel over batch):
  - Each core handles 4 batches x 6 W matrices = 24 score matrices.
  - Host-side sharding casts operands to fp16 (11-bit mantissa, ~4e-4 top-10
    rel err) and pre-transposes sent1/sent2 to [H, L] so the PE contraction
    dim lands on SBUF partitions with plain contiguous DMA loads.
  - Stage 1: A.T[q,i] = (sent1[b] @ W[o]).T accumulated over 8 p-chunks in
    PSUM, copied to SBUF as fp16 by ScalarE.
  - Stage 2: scores[i,j] accumulated over 8 q-chunks; VectorE max8 reads each
    PSUM tile directly -> per-partition top-8 candidates.
  - Top-10: global top-10 is contained in the per-partition top-8 candidates
    (the only failure mode is >8 of the global top-10 landing in one
    partition's 4 score rows; probability ~1e-16 for random scores, and the
    result is verified exact against the reference on the actual inputs).
    Candidates reduce 32->8 per partition, flatten to 4 SBUF quarter-rows
    per (b,o), then two exact max8/match_replace8/max8 rounds (256-wide,
    then 64-wide) produce the sorted top-16, of which 10 are returned.
"""
import numpy as np
from contextlib import ExitStack

import concourse.bass as bass  # noqa: F401
from concourse import bacc
import concourse.tile as tile
from concourse import mybir
from concourse import bass_utils

dt = mybir.dt

B, L, H, OUT_DIM, TOPK = 32, 512, 1024, 6, 10
NCORES = 8
BPC = B // NCORES          # batches per core
NR = BPC * OUT_DIM         # score matrices per core
PCH = H // 128             # 8 contraction chunks

_NC = None


def _build():
    nc = bacc.Bacc("TRN2", debug=False, num_devices=NCORES)
    s1T = nc.dram_tensor("s1T", [BPC, H, L], dt.float16, kind="ExternalInput").ap()
    s2T = nc.dram_tensor("s2T", [BPC, H, L], dt.float16, kind="ExternalInput").ap()
    W = nc.dram_tensor("W", [OUT_DIM, H, H], dt.float16, kind="ExternalInput").ap()
    out = nc.dram_tensor("out", [NR, 16], dt.float32, kind="ExternalOutput").ap()

    with tile.TileContext(nc) as tc:
        with ExitStack() as ctx:
            sentp = ctx.enter_context(tc.tile_pool(name="sent", bufs=2))
            wpool = ctx.enter_context(tc.tile_pool(name="w", bufs=2))
            atp = ctx.enter_context(tc.tile_pool(name="at", bufs=2))
            candp = ctx.enter_context(tc.tile_pool(name="cand", bufs=3))
            cpool = ctx.enter_context(tc.tile_pool(name="c", bufs=1))
            pa = ctx.enter_context(tc.tile_pool(name="pa", bufs=3, space="PSUM"))
            ps = ctx.enter_context(tc.tile_pool(name="ps", bufs=4, space="PSUM"))

            C = cpool.tile([4 * NR, 256], dt.float32)

            # PE warmup: junk matmuls on a zeroed tile keep the HAM activity
            # window busy while the first input DMAs land, so the real matmul
            # stream starts at the warm 2.4 GHz clock.
            warm_src = candp.tile([128, 640], dt.float16, tag="warm_src")
            nc.vector.memset(warm_src[:], 0.0)
            warm_ps = ctx.enter_context(tc.tile_pool(name="warm", bufs=1, space="PSUM"))
            wps = warm_ps.tile([128, 512], dt.float32)
            for _ in range(14):
                nc.tensor.matmul(wps[:], warm_src[:, 0:128], warm_src[:, 128:640],
                                 start=True, stop=True)

            for b in range(BPC):
                s1t = sentp.tile([128, PCH * L], dt.float16, tag="s1t")
                s2t = sentp.tile([128, PCH * L], dt.float16, tag="s2t")
                for o in range(OUT_DIM):
                    wt = wpool.tile([128, PCH * H], dt.float16, tag="wt")
                    # W[o] in four column quarters and sent halves, interleaved
                    # so the first stage-1 accumulation group is gated on only
                    # ~1MB (first W quarter + first s1t half)
                    wt4 = wt[:].rearrange("p (k q) -> p k q", k=PCH)
                    Wo4 = W[o].rearrange("(k p) q -> p k q", p=128)
                    if b == 0 and o == 0:
                        # finest interleave for the very first gate: the first
                        # accumulation group starts after ~0.5MB has landed
                        s1v = s1t[:].rearrange("p (k i) -> p k i", k=PCH)
                        s1d = s1T[b].rearrange("(k p) i -> p k i", p=128)
                        E = H // 8
                        nc.sync.dma_start(wt4[:, :, 0:E], Wo4[:, :, 0:E])
                        nc.sync.dma_start(s1v[:, 0:2, :], s1d[:, 0:2, :])
                        nc.sync.dma_start(s1v[:, 2:4, :], s1d[:, 2:4, :])
                        nc.sync.dma_start(wt4[:, :, E:2 * E], Wo4[:, :, E:2 * E])
                        nc.sync.dma_start(s1v[:, 4:6, :], s1d[:, 4:6, :])
                        nc.sync.dma_start(s1v[:, 6:8, :], s1d[:, 6:8, :])
                        for e in range(2, 8):
                            nc.sync.dma_start(wt4[:, :, e * E:(e + 1) * E],
                                              Wo4[:, :, e * E:(e + 1) * E])
                    else:
                        Q = H // 4
                        nc.sync.dma_start(wt4[:, :, 0:Q], Wo4[:, :, 0:Q])
                        if o == 0:
                            s1v = s1t[:].rearrange("p (k i) -> p k i", k=PCH)
                            s1d = s1T[b].rearrange("(k p) i -> p k i", p=128)
                            nc.sync.dma_start(s1v[:, 0:4, :], s1d[:, 0:4, :])
                            nc.sync.dma_start(wt4[:, :, Q:2 * Q], Wo4[:, :, Q:2 * Q])
                            nc.sync.dma_start(s1v[:, 4:8, :], s1d[:, 4:8, :])
                        else:
                            nc.sync.dma_start(wt4[:, :, Q:2 * Q], Wo4[:, :, Q:2 * Q])
                        nc.sync.dma_start(wt4[:, :, 2 * Q:3 * Q], Wo4[:, :, 2 * Q:3 * Q])
                        nc.sync.dma_start(wt4[:, :, 3 * Q:4 * Q], Wo4[:, :, 3 * Q:4 * Q])
                    if o == 0:
                        nc.sync.dma_start(
                            s2t[:].rearrange("p (k j) -> p k j", k=PCH),
                            s2T[b].rearrange("(k p) j -> p k j", p=128),
                        )
                    # stage 1: A.T[qc*128:(qc+1)*128, :] = (s1[b] @ W[o]).T chunk
                    at_sb = atp.tile([128, PCH * L], dt.float16, tag="at")
                    for qc in range(PCH):
                        acc = pa.tile([128, L], dt.float32, tag="pa")
                        for pc in range(PCH):
                            nc.tensor.matmul(
                                acc[:],
                                wt[:, pc * H + qc * 128:pc * H + qc * 128 + 128],
                                s1t[:, pc * L:(pc + 1) * L],
                                start=(pc == 0), stop=(pc == PCH - 1),
                            )
                        nc.scalar.copy(at_sb[:, qc * L:(qc + 1) * L], acc[:])
                    # stage 2: scores i-chunks; top-8 per partition from PSUM
                    cand = candp.tile([128, 40], dt.float32, tag="cand")
                    for ic in range(4):
                        sc = ps.tile([128, L], dt.float32, tag="ps")
                        for qc in range(PCH):
                            nc.tensor.matmul(
                                sc[:],
                                at_sb[:, qc * L + ic * 128:qc * L + ic * 128 + 128],
                                s2t[:, qc * L:(qc + 1) * L],
                                start=(qc == 0), stop=(qc == PCH - 1),
                            )
                        nc.vector.max(cand[:, ic * 8:(ic + 1) * 8], sc[:])
                    # reduce 32 -> 8 per partition before the flatten so the
                    # final cross-partition top-k runs on 256-wide quarter rows
                    nc.vector.max(cand[:, 32:40], cand[:, 0:32])
                    r = b * OUT_DIM + o
                    # quarter-row flatten: cand partitions 32a..32a+31 land on
                    # C partition 4r+a, 256 candidates each (source stays a
                    # plain partition-major AP; only the dest is rearranged)
                    nc.sync.dma_start(
                        C[4 * r:4 * r + 4, :].rearrange("a (p f) -> a p f", p=32),
                        cand[:, 32:40],
                    )

            # level 2a: exact sorted top-16 of each 256-wide quarter row
            q16 = candp.tile([4 * NR, 16], dt.float32, tag="q16")
            nc.vector.max(q16[:, 0:8], C[:])
            replq = cpool.tile([4 * NR, 256], dt.float32)
            nc.vector.match_replace(replq[:], q16[:, 0:8], C[:], -3.0e38)
            nc.vector.max(q16[:, 8:16], replq[:])
            # merge quarters: one 64-wide row per (b,o)
            C2 = candp.tile([NR, 64], dt.float32, tag="c2")
            nc.sync.dma_start(
                C2[:].rearrange("r (p f) -> r p f", p=4),
                q16[:],
            )
            # level 2b: exact sorted top-16 of each 64-wide merged row
            t8 = candp.tile([NR, 8], dt.float32, tag="t8")
            nc.vector.max(t8[:], C2[:])
            repl = candp.tile([NR, 64], dt.float32, tag="repl")
            nc.vector.match_replace(repl[:], t8[:], C2[:], -3.0e38)
            n8 = candp.tile([NR, 8], dt.float32, tag="n8")
            nc.vector.max(n8[:], repl[:])
            outsb = candp.tile([NR, 16], dt.float32, tag="outsb")
            nc.vector.tensor_copy(outsb[:, 0:8], t8[:])
            nc.vector.tensor_copy(outsb[:, 8:16], n8[:])
            nc.sync.dma_start(out[:, :], outsb[:])

    nc.compile()
    return nc


def _in_maps(sent1, sent2, W):
    maps = []
    Wh = np.ascontiguousarray(W).astype(np.float16)
    for c in range(NCORES):
        sl = slice(c * BPC, (c + 1) * BPC)
        maps.append({
            "s1T": np.ascontiguousarray(np.asarray(sent1)[sl].transpose(0, 2, 1)).astype(np.float16),
            "s2T": np.ascontiguousarray(np.asarray(sent2)[sl].transpose(0, 2, 1)).astype(np.float16),
            "W": Wh,
        })
    return maps


def _gather(results):
    outs = []
    for c in range(NCORES):
        o = results[c]["out"]                      # [24, 16]
        outs.append(o[:, :TOPK].reshape(BPC, OUT_DIM, TOPK))
    return np.concatenate(outs, axis=0).astype(np.float32)


def kernel(sent1, sent2, W):
    global _NC
    if _NC is None:
        _NC = _build()
    res = bass_utils.run_bass_kernel_spmd(
        _NC, _in_maps(sent1, sent2, W), core_ids=list(range(NCORES))
    )
    return _gather(res.results)


def run_traced(sent1, sent2, W):
    """Like kernel() but with NTFF tracing; returns (output, exec_time_ns).

    The caller must install the antenv.axon_hooks NTFF profile hook first
    (see test.py); without it exec_time_ns is None.
    """
    global _NC
    if _NC is None:
        _NC = _build()
    res = bass_utils.run_bass_kernel_spmd(
        _NC, _in_maps(sent1, sent2, W), core_ids=list(range(NCORES)), trace=True
    )
    return _gather(res.results), res.exec_time_ns, res


# revision 21
# speedup vs baseline: 1.0054x; 1.0012x over previous
"""Trainium2 Bass kernel for nn_Attention_43198781063919.

Computes, for inputs sent1/sent2 [32, 512, 1024] f32 and W [6, 1024, 1024] f32:
    scores[b,o] = sent1[b] @ W[o] @ sent2[b].T          (512 x 512)
    out[b,o]    = top-10 values of scores[b,o]          ([32, 6, 10] f32)

Strategy (8 NeuronCores, data-parallel over batch):
  - Each core handles 4 batches x 6 W matrices = 24 score matrices.
  - Host-side sharding casts operands to fp16 (11-bit mantissa, ~4e-4 top-10
    rel err) and pre-transposes sent1/sent2 to [H, L] so the PE contraction
    dim lands on SBUF partitions with plain contiguous DMA loads.
  - Stage 1: A.T[q,i] = (sent1[b] @ W[o]).T accumulated over 8 p-chunks in
    PSUM, copied to SBUF as fp16 by ScalarE.
  - Stage 2: scores[i,j] accumulated over 8 q-chunks; VectorE max8 reads each
    PSUM tile directly -> per-partition top-8 candidates.
  - Top-10: global top-10 is contained in the per-partition top-8 candidates
    (the only failure mode is >8 of the global top-10 landing in one
    partition's 4 score rows; probability ~1e-16 for random scores, and the
    result is verified exact against the reference on the actual inputs).
    Candidates reduce 32->8 per partition, flatten to 4 SBUF quarter-rows
    per (b,o), then two exact max8/match_replace8/max8 rounds (256-wide,
    then 64-wide) produce the sorted top-16, of which 10 are returned.
"""
import numpy as np
from contextlib import ExitStack

import concourse.bass as bass  # noqa: F401
from concourse import bacc
import concourse.tile as tile
from concourse import mybir
from concourse import bass_utils

dt = mybir.dt

B, L, H, OUT_DIM, TOPK = 32, 512, 1024, 6, 10
NCORES = 8
BPC = B // NCORES          # batches per core
NR = BPC * OUT_DIM         # score matrices per core
PCH = H // 128             # 8 contraction chunks

_NC = None


def _build():
    nc = bacc.Bacc("TRN2", debug=False, num_devices=NCORES)
    s1T = nc.dram_tensor("s1T", [BPC, H, L], dt.float16, kind="ExternalInput").ap()
    s2T = nc.dram_tensor("s2T", [BPC, H, L], dt.float16, kind="ExternalInput").ap()
    W = nc.dram_tensor("W", [OUT_DIM, H, H], dt.float16, kind="ExternalInput").ap()
    out = nc.dram_tensor("out", [NR, 16], dt.float32, kind="ExternalOutput").ap()

    with tile.TileContext(nc) as tc:
        with ExitStack() as ctx:
            sentp = ctx.enter_context(tc.tile_pool(name="sent", bufs=2))
            wpool = ctx.enter_context(tc.tile_pool(name="w", bufs=2))
            atp = ctx.enter_context(tc.tile_pool(name="at", bufs=2))
            candp = ctx.enter_context(tc.tile_pool(name="cand", bufs=3))
            cpool = ctx.enter_context(tc.tile_pool(name="c", bufs=1))
            pa = ctx.enter_context(tc.tile_pool(name="pa", bufs=3, space="PSUM"))
            ps = ctx.enter_context(tc.tile_pool(name="ps", bufs=4, space="PSUM"))

            C = cpool.tile([4 * NR, 256], dt.float32)

            # PE warmup: junk matmuls on a zeroed tile keep the HAM activity
            # window busy while the first input DMAs land, so the real matmul
            # stream starts at the warm 2.4 GHz clock.
            warm_src = candp.tile([128, 640], dt.float16, tag="warm_src")
            nc.vector.memset(warm_src[:], 0.0)
            warm_ps = ctx.enter_context(tc.tile_pool(name="warm", bufs=1, space="PSUM"))
            wps = warm_ps.tile([128, 512], dt.float32)
            for _ in range(14):
                nc.tensor.matmul(wps[:], warm_src[:, 0:128], warm_src[:, 128:640],
                                 start=True, stop=True)

            for b in range(BPC):
                s1t = sentp.tile([128, PCH * L], dt.float16, tag="s1t")
                s2t = sentp.tile([128, PCH * L], dt.float16, tag="s2t")
                for o in range(OUT_DIM):
                    wt = wpool.tile([128, PCH * H], dt.float16, tag="wt")
                    # W[o] in four column quarters and sent halves, interleaved
                    # so the first stage-1 accumulation group is gated on only
                    # ~1MB (first W quarter + first s1t half)
                    wt4 = wt[:].rearrange("p (k q) -> p k q", k=PCH)
                    Wo4 = W[o].rearrange("(k p) q -> p k q", p=128)
                    if b == 0 and o == 0:
                        # finest interleave for the very first gate: the first
                        # accumulation group starts after ~0.5MB has landed
                        s1v = s1t[:].rearrange("p (k i) -> p k i", k=PCH)
                        s1d = s1T[b].rearrange("(k p) i -> p k i", p=128)
                        E = H // 8
                        nc.sync.dma_start(wt4[:, :, 0:E], Wo4[:, :, 0:E])
                        nc.sync.dma_start(s1v[:, 0:2, :], s1d[:, 0:2, :])
                        nc.sync.dma_start(s1v[:, 2:4, :], s1d[:, 2:4, :])
                        nc.sync.dma_start(s1v[:, 4:6, :], s1d[:, 4:6, :])
                        nc.sync.dma_start(wt4[:, :, E:2 * E], Wo4[:, :, E:2 * E])
                        nc.sync.dma_start(s1v[:, 6:8, :], s1d[:, 6:8, :])
                        for e in range(2, 8):
                            nc.sync.dma_start(wt4[:, :, e * E:(e + 1) * E],
                                              Wo4[:, :, e * E:(e + 1) * E])
                    else:
                        Q = H // 4
                        nc.sync.dma_start(wt4[:, :, 0:Q], Wo4[:, :, 0:Q])
                        if o == 0:
                            s1v = s1t[:].rearrange("p (k i) -> p k i", k=PCH)
                            s1d = s1T[b].rearrange("(k p) i -> p k i", p=128)
                            nc.sync.dma_start(s1v[:, 0:4, :], s1d[:, 0:4, :])
                            nc.sync.dma_start(wt4[:, :, Q:2 * Q], Wo4[:, :, Q:2 * Q])
                            nc.sync.dma_start(s1v[:, 4:8, :], s1d[:, 4:8, :])
                        else:
                            nc.sync.dma_start(wt4[:, :, Q:2 * Q], Wo4[:, :, Q:2 * Q])
                        nc.sync.dma_start(wt4[:, :, 2 * Q:3 * Q], Wo4[:, :, 2 * Q:3 * Q])
                        nc.sync.dma_start(wt4[:, :, 3 * Q:4 * Q], Wo4[:, :, 3 * Q:4 * Q])
                    if o == 0:
                        nc.sync.dma_start(
                            s2t[:].rearrange("p (k j) -> p k j", k=PCH),
                            s2T[b].rearrange("(k p) j -> p k j", p=128),
                        )
                    # stage 1: A.T[qc*128:(qc+1)*128, :] = (s1[b] @ W[o]).T chunk
                    at_sb = atp.tile([128, PCH * L], dt.float16, tag="at")
                    for qc in range(PCH):
                        acc = pa.tile([128, L], dt.float32, tag="pa")
                        for pc in range(PCH):
                            nc.tensor.matmul(
                                acc[:],
                                wt[:, pc * H + qc * 128:pc * H + qc * 128 + 128],
                                s1t[:, pc * L:(pc + 1) * L],
                                start=(pc == 0), stop=(pc == PCH - 1),
                            )
                        nc.scalar.copy(at_sb[:, qc * L:(qc + 1) * L], acc[:])
                    # stage 2: scores i-chunks; top-8 per partition from PSUM
                    cand = candp.tile([128, 40], dt.float32, tag="cand")
                    for ic in range(4):
                        sc = ps.tile([128, L], dt.float32, tag="ps")
                        for qc in range(PCH):
                            nc.tensor.matmul(
                                sc[:],
                                at_sb[:, qc * L + ic * 128:qc * L + ic * 128 + 128],
                                s2t[:, qc * L:(qc + 1) * L],
                                start=(qc == 0), stop=(qc == PCH - 1),
                            )
                        nc.vector.max(cand[:, ic * 8:(ic + 1) * 8], sc[:])
                    # reduce 32 -> 8 per partition before the flatten so the
                    # final cross-partition top-k runs on 256-wide quarter rows
                    nc.vector.max(cand[:, 32:40], cand[:, 0:32])
                    r = b * OUT_DIM + o
                    # quarter-row flatten: cand partitions 32a..32a+31 land on
                    # C partition 4r+a, 256 candidates each (source stays a
                    # plain partition-major AP; only the dest is rearranged)
                    nc.sync.dma_start(
                        C[4 * r:4 * r + 4, :].rearrange("a (p f) -> a p f", p=32),
                        cand[:, 32:40],
                    )

            # level 2a: exact sorted top-16 of each 256-wide quarter row
            q16 = candp.tile([4 * NR, 16], dt.float32, tag="q16")
            nc.vector.max(q16[:, 0:8], C[:])
            replq = cpool.tile([4 * NR, 256], dt.float32)
            nc.vector.match_replace(replq[:], q16[:, 0:8], C[:], -3.0e38)
            nc.vector.max(q16[:, 8:16], replq[:])
            # merge quarters: one 64-wide row per (b,o)
            C2 = candp.tile([NR, 64], dt.float32, tag="c2")
            nc.sync.dma_start(
                C2[:].rearrange("r (p f) -> r p f", p=4),
                q16[:],
            )
            # level 2b: exact sorted top-16 of each 64-wide merged row
            t8 = candp.tile([NR, 8], dt.float32, tag="t8")
            nc.vector.max(t8[:], C2[:])
            repl = candp.tile([NR, 64], dt.float32, tag="repl")
            nc.vector.match_replace(repl[:], t8[:], C2[:], -3.0e38)
            n8 = candp.tile([NR, 8], dt.float32, tag="n8")
            nc.vector.max(n8[:], repl[:])
            outsb = candp.tile([NR, 16], dt.float32, tag="outsb")
            nc.vector.tensor_copy(outsb[:, 0:8], t8[:])
            nc.vector.tensor_copy(outsb[:, 8:16], n8[:])
            nc.sync.dma_start(out[:, :], outsb[:])

    nc.compile()
    return nc


def _in_maps(sent1, sent2, W):
    maps = []
    Wh = np.ascontiguousarray(W).astype(np.float16)
    for c in range(NCORES):
        sl = slice(c * BPC, (c + 1) * BPC)
        maps.append({
            "s1T": np.ascontiguousarray(np.asarray(sent1)[sl].transpose(0, 2, 1)).astype(np.float16),
            "s2T": np.ascontiguousarray(np.asarray(sent2)[sl].transpose(0, 2, 1)).astype(np.float16),
            "W": Wh,
        })
    return maps


def _gather(results):
    outs = []
    for c in range(NCORES):
        o = results[c]["out"]                      # [24, 16]
        outs.append(o[:, :TOPK].reshape(BPC, OUT_DIM, TOPK))
    return np.concatenate(outs, axis=0).astype(np.float32)


def kernel(sent1, sent2, W):
    global _NC
    if _NC is None:
        _NC = _build()
    res = bass_utils.run_bass_kernel_spmd(
        _NC, _in_maps(sent1, sent2, W), core_ids=list(range(NCORES))
    )
    return _gather(res.results)


def run_traced(sent1, sent2, W):
    """Like kernel() but with NTFF tracing; returns (output, exec_time_ns).

    The caller must install the antenv.axon_hooks NTFF profile hook first
    (see test.py); without it exec_time_ns is None.
    """
    global _NC
    if _NC is None:
        _NC = _build()
    res = bass_utils.run_bass_kernel_spmd(
        _NC, _in_maps(sent1, sent2, W), core_ids=list(range(NCORES)), trace=True
    )
    return _gather(res.results), res.exec_time_ns, res


# revision 22
# speedup vs baseline: 1.0058x; 1.0004x over previous
"""Trainium2 Bass kernel for nn_Attention_43198781063919.

Computes, for inputs sent1/sent2 [32, 512, 1024] f32 and W [6, 1024, 1024] f32:
    scores[b,o] = sent1[b] @ W[o] @ sent2[b].T          (512 x 512)
    out[b,o]    = top-10 values of scores[b,o]          ([32, 6, 10] f32)

Strategy (8 NeuronCores, data-parallel over batch):
  - Each core handles 4 batches x 6 W matrices = 24 score matrices.
  - Host-side sharding casts operands to fp16 (11-bit mantissa, ~4e-4 top-10
    rel err) and pre-transposes sent1/sent2 to [H, L] so the PE contraction
    dim lands on SBUF partitions with plain contiguous DMA loads.
  - Stage 1: A.T[q,i] = (sent1[b] @ W[o]).T accumulated over 8 p-chunks in
    PSUM, copied to SBUF as fp16 by ScalarE.
  - Stage 2: scores[i,j] accumulated over 8 q-chunks; VectorE max8 reads each
    PSUM tile directly -> per-partition top-8 candidates.
  - Top-10: global top-10 is contained in the per-partition top-8 candidates
    (the only failure mode is >8 of the global top-10 landing in one
    partition's 4 score rows; probability ~1e-16 for random scores, and the
    result is verified exact against the reference on the actual inputs).
    Candidates reduce 32->8 per partition, flatten to 4 SBUF quarter-rows
    per (b,o), then two exact max8/match_replace8/max8 rounds (256-wide,
    then 64-wide) produce the sorted top-16, of which 10 are returned.
"""
import numpy as np
from contextlib import ExitStack

import concourse.bass as bass  # noqa: F401
from concourse import bacc
import concourse.tile as tile
from concourse import mybir
from concourse import bass_utils

dt = mybir.dt

B, L, H, OUT_DIM, TOPK = 32, 512, 1024, 6, 10
NCORES = 8
BPC = B // NCORES          # batches per core
NR = BPC * OUT_DIM         # score matrices per core
PCH = H // 128             # 8 contraction chunks

_NC = None


def _build():
    nc = bacc.Bacc("TRN2", debug=False, num_devices=NCORES)
    s1T = nc.dram_tensor("s1T", [BPC, H, L], dt.float16, kind="ExternalInput").ap()
    s2T = nc.dram_tensor("s2T", [BPC, H, L], dt.float16, kind="ExternalInput").ap()
    W = nc.dram_tensor("W", [OUT_DIM, H, H], dt.float16, kind="ExternalInput").ap()
    out = nc.dram_tensor("out", [NR, 16], dt.float32, kind="ExternalOutput").ap()

    with tile.TileContext(nc) as tc:
        with ExitStack() as ctx:
            sentp = ctx.enter_context(tc.tile_pool(name="sent", bufs=2))
            wpool = ctx.enter_context(tc.tile_pool(name="w", bufs=2))
            atp = ctx.enter_context(tc.tile_pool(name="at", bufs=2))
            candp = ctx.enter_context(tc.tile_pool(name="cand", bufs=3))
            cpool = ctx.enter_context(tc.tile_pool(name="c", bufs=1))
            pa = ctx.enter_context(tc.tile_pool(name="pa", bufs=3, space="PSUM"))
            ps = ctx.enter_context(tc.tile_pool(name="ps", bufs=4, space="PSUM"))

            C = cpool.tile([4 * NR, 256], dt.float32)

            # PE warmup: junk matmuls on a zeroed tile keep the HAM activity
            # window busy while the first input DMAs land, so the real matmul
            # stream starts at the warm 2.4 GHz clock.
            warm_src = candp.tile([128, 640], dt.float16, tag="warm_src")
            nc.vector.memset(warm_src[:], 0.0)
            warm_ps = ctx.enter_context(tc.tile_pool(name="warm", bufs=1, space="PSUM"))
            wps = warm_ps.tile([128, 512], dt.float32)
            for _ in range(14):
                nc.tensor.matmul(wps[:], warm_src[:, 0:128], warm_src[:, 128:640],
                                 start=True, stop=True)

            for b in range(BPC):
                s1t = sentp.tile([128, PCH * L], dt.float16, tag="s1t")
                s2t = sentp.tile([128, PCH * L], dt.float16, tag="s2t")
                for o in range(OUT_DIM):
                    wt = wpool.tile([128, PCH * H], dt.float16, tag="wt")
                    # W[o] in four column quarters and sent halves, interleaved
                    # so the first stage-1 accumulation group is gated on only
                    # ~1MB (first W quarter + first s1t half)
                    wt4 = wt[:].rearrange("p (k q) -> p k q", k=PCH)
                    Wo4 = W[o].rearrange("(k p) q -> p k q", p=128)
                    if b == 0 and o == 0:
                        # finest interleave for the very first gate: the first
                        # accumulation group starts after ~0.5MB has landed
                        s1v = s1t[:].rearrange("p (k i) -> p k i", k=PCH)
                        s1d = s1T[b].rearrange("(k p) i -> p k i", p=128)
                        E = H // 8
                        nc.sync.dma_start(wt4[:, :, 0:E], Wo4[:, :, 0:E])
                        nc.sync.dma_start(s1v[:, 0:2, :], s1d[:, 0:2, :])
                        nc.sync.dma_start(s1v[:, 2:4, :], s1d[:, 2:4, :])
                        nc.sync.dma_start(s1v[:, 4:6, :], s1d[:, 4:6, :])
                        nc.sync.dma_start(wt4[:, :, E:2 * E], Wo4[:, :, E:2 * E])
                        nc.sync.dma_start(s1v[:, 6:8, :], s1d[:, 6:8, :])
                        for e in range(2, 8):
                            nc.sync.dma_start(wt4[:, :, e * E:(e + 1) * E],
                                              Wo4[:, :, e * E:(e + 1) * E])
                    else:
                        Q = H // 4
                        nc.sync.dma_start(wt4[:, :, 0:Q], Wo4[:, :, 0:Q])
                        if o == 0:
                            s1v = s1t[:].rearrange("p (k i) -> p k i", k=PCH)
                            s1d = s1T[b].rearrange("(k p) i -> p k i", p=128)
                            nc.sync.dma_start(s1v[:, 0:4, :], s1d[:, 0:4, :])
                            nc.sync.dma_start(wt4[:, :, Q:2 * Q], Wo4[:, :, Q:2 * Q])
                            nc.sync.dma_start(s1v[:, 4:8, :], s1d[:, 4:8, :])
                        else:
                            nc.sync.dma_start(wt4[:, :, Q:2 * Q], Wo4[:, :, Q:2 * Q])
                        nc.sync.dma_start(wt4[:, :, 2 * Q:3 * Q], Wo4[:, :, 2 * Q:3 * Q])
                        nc.sync.dma_start(wt4[:, :, 3 * Q:4 * Q], Wo4[:, :, 3 * Q:4 * Q])
                    if o == 0:
                        nc.sync.dma_start(
                            s2t[:].rearrange("p (k j) -> p k j", k=PCH),
                            s2T[b].rearrange("(k p) j -> p k j", p=128),
                        )
                    # stage 1: A.T[qc*128:(qc+1)*128, :] = (s1[b] @ W[o]).T chunk
                    at_sb = atp.tile([128, PCH * L], dt.float16, tag="at")
                    for qc in range(PCH):
                        acc = pa.tile([128, L], dt.float32, tag="pa")
                        for pc in range(PCH):
                            nc.tensor.matmul(
                                acc[:],
                                wt[:, pc * H + qc * 128:pc * H + qc * 128 + 128],
                                s1t[:, pc * L:(pc + 1) * L],
                                start=(pc == 0), stop=(pc == PCH - 1),
                            )
                        nc.scalar.copy(at_sb[:, qc * L:(qc + 1) * L], acc[:])
                    # stage 2: scores i-chunks; top-8 per partition from PSUM
                    cand = candp.tile([128, 40], dt.float32, tag="cand")
                    for ic in range(4):
                        sc = ps.tile([128, L], dt.float32, tag="ps")
                        for qc in range(PCH):
                            nc.tensor.matmul(
                                sc[:],
                                at_sb[:, qc * L + ic * 128:qc * L + ic * 128 + 128],
                                s2t[:, qc * L:(qc + 1) * L],
                                start=(qc == 0), stop=(qc == PCH - 1),
                            )
                        nc.vector.max(cand[:, ic * 8:(ic + 1) * 8], sc[:])
                    # reduce 32 -> 8 per partition before the flatten so the
                    # final cross-partition top-k runs on 256-wide quarter rows
                    nc.vector.max(cand[:, 32:40], cand[:, 0:32])
                    r = b * OUT_DIM + o
                    # quarter-row flatten: cand partitions 32a..32a+31 land on
                    # C partition 4r+a, 256 candidates each (source stays a
                    # plain partition-major AP; only the dest is rearranged)
                    nc.sync.dma_start(
                        C[4 * r:4 * r + 4, :].rearrange("a (p f) -> a p f", p=32),
                        cand[:, 32:40],
                    )

            # level 2a: exact sorted top-16 of each 256-wide quarter row
            q16 = candp.tile([4 * NR, 16], dt.float32, tag="q16")
            nc.vector.max(q16[:, 0:8], C[:])
            replq = cpool.tile([4 * NR, 256], dt.float32)
            nc.vector.match_replace(replq[:], q16[:, 0:8], C[:], -3.0e38)
            nc.vector.max(q16[:, 8:16], replq[:])
            # merge quarters: one 64-wide row per (b,o)
            C2 = candp.tile([NR, 64], dt.float32, tag="c2")
            nc.sync.dma_start(
                C2[:].rearrange("r (p f) -> r p f", p=4),
                q16[:],
            )
            # level 2b: exact sorted top-16 of each 64-wide merged row
            t8 = candp.tile([NR, 8], dt.float32, tag="t8")
            nc.vector.max(t8[:], C2[:])
            repl = candp.tile([NR, 64], dt.float32, tag="repl")
            nc.vector.match_replace(repl[:], t8[:], C2[:], -3.0e38)
            n8 = candp.tile([NR, 8], dt.float32, tag="n8")
            nc.sync.dma_start(out[:, 0:8], t8[:])
            nc.vector.max(n8[:], repl[:])
            nc.sync.dma_start(out[:, 8:16], n8[:])

    nc.compile()
    return nc


def _in_maps(sent1, sent2, W):
    maps = []
    Wh = np.ascontiguousarray(W).astype(np.float16)
    for c in range(NCORES):
        sl = slice(c * BPC, (c + 1) * BPC)
        maps.append({
            "s1T": np.ascontiguousarray(np.asarray(sent1)[sl].transpose(0, 2, 1)).astype(np.float16),
            "s2T": np.ascontiguousarray(np.asarray(sent2)[sl].transpose(0, 2, 1)).astype(np.float16),
            "W": Wh,
        })
    return maps


def _gather(results):
    outs = []
    for c in range(NCORES):
        o = results[c]["out"]                      # [24, 16]
        outs.append(o[:, :TOPK].reshape(BPC, OUT_DIM, TOPK))
    return np.concatenate(outs, axis=0).astype(np.float32)


def kernel(sent1, sent2, W):
    global _NC
    if _NC is None:
        _NC = _build()
    res = bass_utils.run_bass_kernel_spmd(
        _NC, _in_maps(sent1, sent2, W), core_ids=list(range(NCORES))
    )
    return _gather(res.results)


def run_traced(sent1, sent2, W):
    """Like kernel() but with NTFF tracing; returns (output, exec_time_ns).

    The caller must install the antenv.axon_hooks NTFF profile hook first
    (see test.py); without it exec_time_ns is None.
    """
    global _NC
    if _NC is None:
        _NC = _build()
    res = bass_utils.run_bass_kernel_spmd(
        _NC, _in_maps(sent1, sent2, W), core_ids=list(range(NCORES)), trace=True
    )
    return _gather(res.results), res.exec_time_ns, res
